# revision 14
# baseline (speedup 1.0000x reference)
"""KronEmbedding lookup kernel for 8 TRN2 NeuronCores.

Math: w = einsum('sia,sjb->ijab', A, B).reshape(50176, 2048); out = w[x].
Never materializes w. Per token t with i=x//224, j=x%224:
    out[t] = sum_s outer(A[s,i,:], B[s,j,:])   -> (64*32 = 2048 floats)

Strategy (data-parallel over tokens, 1024 tokens/core, all-bf16 device
compute; tolerance 2e-2 >> bf16 rounding):
- Host: token-major bf16 tables A4[i] = A[:,i,:] (512 vals, s-major) and
  B4[j] = B[:,j,:] (256 vals); per-core idx arrays in SWDGE wrapped int16.
- Device per core, pipelined in 2 halves (4 tiles of 128 tokens each):
  * SWDGE gather, ONE ROW PER TOKEN (8x fewer Q7 row descriptors than
    per-(token,s) gathers; gather cost is per-row, not per-byte).
  * Partition shuffle token-major -> contraction layout
    Ag[(16s+k) part, g, c, .] (token t = 8k+g within tile c) via a DRAM
    round trip: 8 s-block writes (write address is linear in the source
    partition, so the AP is 2-dim) + 1 full readback per operand.
    SBUF->SBUF DMAs cannot cross partition structures; DRAM-hop DMAs can.
  * Block-diag moving operand via DVE broadcast x static 0/1 mask:
    BD[p, (k',b)] = Bg[p, b] * (k' == p%16). No per-piece descriptors.
  * Per (tile c, group g): matmul psum[64,512] = Ag[:,g,c,:]^T @ BD.
    4 groups packed per [128,1024] psum tile; 2 psum tiles per tile c.
  * Evac psum -> bf16 SBUF (ACT + some DVE), one 512 KB out-DMA per tile
    (split across sync/scalar queues).
- Host: upcast bf16 -> f32, unshuffle token/emb order.
"""
import numpy as np
import ml_dtypes
from contextlib import ExitStack

import concourse.bass as bass
import concourse.bacc as bacc
import concourse.tile as tile
import concourse.mybir as mybir
from concourse import bass_utils

dt = mybir.dt
bf16 = ml_dtypes.bfloat16

R, M1, N1, M2, N2 = 8, 224, 64, 224, 32
VOCAB, EMB = M1 * M2, N1 * N2          # 50176, 2048
BATCH, SEQ = 4, 2048
NTOK = BATCH * SEQ                     # 8192
NCORES = 8
TPC = NTOK // NCORES                   # 1024 tokens per core
NTILES = TPC // 128                    # 8 tiles of 128 tokens
NG = 8                                 # groups per tile (token t = 8k+g)

_CACHE = {}


def _build():
    nc = bacc.Bacc("TRN2", num_devices=NCORES)
    A4 = nc.dram_tensor("A4", [M1, 512], dt.bfloat16, kind="ExternalInput")
    B4 = nc.dram_tensor("B4", [M2, 256], dt.bfloat16, kind="ExternalInput")
    idxA = nc.dram_tensor("idxA", [128, 64], dt.int16, kind="ExternalInput")
    idxB = nc.dram_tensor("idxB", [128, 64], dt.int16, kind="ExternalInput")
    maskT = nc.dram_tensor("maskT", [128, 512], dt.bfloat16, kind="ExternalInput")
    # DRAM scratch for the partition-shuffle round trip (host ignores);
    # separate tensors per half so Tile sees no false cross-half deps.
    Asc = [
        nc.dram_tensor(f"Asc{h}", [8, 16, 8, 4, 64], dt.bfloat16,
                       kind="ExternalOutput")
        for h in range(2)
    ]
    Bsc = [
        nc.dram_tensor(f"Bsc{h}", [8, 16, 8, 4, 32], dt.bfloat16,
                       kind="ExternalOutput")
        for h in range(2)
    ]
    out = nc.dram_tensor(
        "out", [NTILES, 128, 2048], dt.bfloat16, kind="ExternalOutput"
    )

    with tile.TileContext(nc) as tc, ExitStack() as ctx:
        cpool = ctx.enter_context(tc.tile_pool(name="const", bufs=1))
        ahp = ctx.enter_context(tc.tile_pool(name="ah", bufs=2))
        bhp = ctx.enter_context(tc.tile_pool(name="bh", bufs=2))
        agp = ctx.enter_context(tc.tile_pool(name="ag", bufs=2))
        bgp = ctx.enter_context(tc.tile_pool(name="bg", bufs=2))
        bdp = ctx.enter_context(tc.tile_pool(name="bd", bufs=8))
        psp = ctx.enter_context(tc.tile_pool(name="ps", bufs=4, space="PSUM"))
        evp = ctx.enter_context(tc.tile_pool(name="ev", bufs=4))

        idxA_sb = cpool.tile([128, 64], dt.int16, tag="idxA")
        idxB_sb = cpool.tile([128, 64], dt.int16, tag="idxB")
        mask_sb = cpool.tile([128, 512], dt.bfloat16, tag="mask")
        nc.sync.dma_start(idxA_sb[:], idxA[:])
        nc.sync.dma_start(idxB_sb[:], idxB[:])
        nc.sync.dma_start(mask_sb[:], maskT[:])

        # per half h (tiles 4h..4h+4, 512 tokens): gather token-major
        # (row idx[c*128+p] lands at out[p, c]), then round-trip shuffle.
        ag, bg = [], []
        for h in range(2):
            bh = bhp.tile([128, 4, 256], dt.bfloat16, tag="bh", name=f"bh{h}")
            nc.gpsimd.dma_gather(
                bh[:], B4[:], idxB_sb[:, 32 * h:32 * h + 32], 512, 512, 256
            )
            ah = ahp.tile([128, 4, 512], dt.bfloat16, tag="ah", name=f"ah{h}")
            nc.gpsimd.dma_gather(
                ah[:], A4[:], idxA_sb[:, 32 * h:32 * h + 32], 512, 512, 512
            )
            for s in range(8):
                nc.scalar.dma_start(
                    Bsc[h][s].rearrange("k g c a -> (k g) (c a)"),
                    bh[:, :, 32 * s:32 * s + 32],
                )
                nc.sync.dma_start(
                    Asc[h][s].rearrange("k g c a -> (k g) (c a)"),
                    ah[:, :, 64 * s:64 * s + 64],
                )
            # readbacks on sync (never behind the other half's writes,
            # which queue on scalar and become ready earlier)
            b = bgp.tile([128, NG, 4, 32], dt.bfloat16, tag="bg", name=f"bg{h}")
            nc.sync.dma_start(b[:], Bsc[h][:].rearrange("s k g c a -> (s k) g c a"))
            a = agp.tile([128, NG, 4, 64], dt.bfloat16, tag="ag", name=f"ag{h}")
            nc.sync.dma_start(a[:], Asc[h][:].rearrange("s k g c a -> (s k) g c a"))
            ag.append(a)
            bg.append(b)

        # per tile: 4 BD builds (DVE, 2 groups each) + 8 matmuls;
        # evac ACT + some DVE; out DMAs split sync/scalar.
        mask3 = mask_sb[:].rearrange("p (k b) -> p k b", k=16)
        for c in range(NTILES):
            h, cc = c // 4, c % 4
            ev = evp.tile([128, 2, 1024], dt.bfloat16, tag="ev")
            for half in range(2):
                ps = psp.tile([128, 1024], dt.float32, tag="ps")
                for qq in range(2):
                    g0 = 4 * half + 2 * qq
                    bdt = bdp.tile([128, 2, 16, 32], dt.bfloat16, tag="bd")
                    src = (
                        bg[h][:, g0:g0 + 2, cc, :]
                        .unsqueeze(2)
                        .broadcast_to([128, 2, 16, 32])
                    )
                    msk = mask3.unsqueeze(1).broadcast_to([128, 2, 16, 32])
                    nc.vector.tensor_mul(bdt[:], src, msk)
                    for j in range(2):
                        q = 2 * qq + j
                        nc.tensor.matmul(
                            ps[64 * (q % 2):64 * (q % 2) + 64,
                               512 * (q // 2):512 * (q // 2) + 512],
                            ag[h][:, g0 + j, cc, :],
                            bdt[:, j, :, :].rearrange("p k b -> p (k b)"),
                            start=True,
                            stop=True,
                        )
                if (2 * c + half) % 4 == 3:
                    nc.vector.tensor_copy(ev[:, half, :], ps[:])
                else:
                    nc.scalar.copy(ev[:, half, :], ps[:])
            oeng = nc.sync if c % 2 == 0 else nc.scalar
            oeng.dma_start(out[c], ev[:].rearrange("p h e -> p (h e)"))

    nc.compile()
    return nc


def _wrap_idxs(idx: np.ndarray) -> np.ndarray:
    """[n] -> SWDGE wrapped layout [128, n//16] int16; gather places
    row idx[c*128+p] at out[p, c]."""
    n = idx.shape[0]
    w = idx.reshape(n // 16, 16).T.astype(np.int16)
    return np.ascontiguousarray(np.tile(w, (8, 1)))


def _in_maps(A, B, x):
    A = np.asarray(A, dtype=np.float32)
    B = np.asarray(B, dtype=np.float32)
    xl = np.asarray(x).astype(np.int64).reshape(-1)           # [8192]

    A4 = np.ascontiguousarray(A.transpose(1, 0, 2).reshape(M1, 512)).astype(bf16)
    B4 = np.ascontiguousarray(B.transpose(1, 0, 2).reshape(M2, 256)).astype(bf16)

    i_all = (xl // M2).astype(np.int64)
    j_all = (xl % M2).astype(np.int64)

    # maskT[p, k'*32+b] = (k' == p % 16)
    mask = (np.arange(16)[None, :, None] == (np.arange(128) % 16)[:, None, None])
    maskT = np.ascontiguousarray(
        np.broadcast_to(mask, (128, 16, 32)).reshape(128, 512).astype(bf16)
    )

    in_maps = []
    for core in range(NCORES):
        sl = slice(core * TPC, (core + 1) * TPC)
        in_maps.append(
            dict(
                A4=A4,
                B4=B4,
                idxA=_wrap_idxs(i_all[sl]),
                idxB=_wrap_idxs(j_all[sl]),
                maskT=maskT,
            )
        )
    return in_maps


def _decode(res):
    outs = []
    for core in range(NCORES):
        o = np.asarray(res.results[core]["out"]).astype(np.float32)
        # out[c, p, inner]: p = 64*rowhalf + a (rowhalf = q%2);
        # inner = 1024*half + 512*colblk + 32*k + b  (g = 4*half+2*colblk+rowhalf)
        o = o.reshape(NTILES, 2, 64, 2, 2, 16, 32)  # c, rh, a, half, cb, k, b
        o = o.transpose(0, 5, 3, 4, 1, 2, 6)        # c, k, half, cb, rh, a, b
        outs.append(o.reshape(TPC, EMB))            # token = c*128 + 8k + g
    full = np.concatenate(outs, axis=0)             # [8192, 2048]
    return full.reshape(BATCH, SEQ, EMB).astype(np.float32)


def kernel(A: np.ndarray, B: np.ndarray, x: np.ndarray) -> np.ndarray:
    if "nc" not in _CACHE:
        _CACHE["nc"] = _build()
    nc = _CACHE["nc"]
    in_maps = _in_maps(A, B, x)
    res = bass_utils.run_bass_kernel_spmd(nc, in_maps, core_ids=list(range(NCORES)))
    return _decode(res)


# revision 15
# speedup vs baseline: 1.1720x; 1.1720x over previous
"""KronEmbedding lookup kernel for 8 TRN2 NeuronCores.

Math: w = einsum('sia,sjb->ijab', A, B).reshape(50176, 2048); out = w[x].
Never materializes w. Per token t with i=x//224, j=x%224:
    out[t] = sum_s outer(A[s,i,:], B[s,j,:])   -> (64*32 = 2048 floats)

Strategy (data-parallel over tokens, 1024 tokens/core, all-bf16 device
compute; tolerance 2e-2 >> bf16 rounding):
- Host: token-major bf16 tables A4[i] = A[:,i,:] (512 vals, s-major) and
  B4[j] = B[:,j,:] (256 vals); per-core idx arrays in SWDGE wrapped int16.
- Device per core, pipelined in 2 halves (4 tiles of 128 tokens each):
  * SWDGE gather, ONE ROW PER TOKEN (8x fewer Q7 row descriptors than
    per-(token,s) gathers; gather cost is per-row, not per-byte).
  * Partition shuffle token-major -> contraction layout
    Ag[(16s+k) part, g, c, .] (token t = 8k+g within tile c) via a DRAM
    round trip: 8 s-block writes (write address is linear in the source
    partition, so the AP is 2-dim) + 1 full readback per operand.
    SBUF->SBUF DMAs cannot cross partition structures; DRAM-hop DMAs can.
  * Block-diag moving operand via DVE broadcast x static 0/1 mask:
    BD[p, (k',b)] = Bg[p, b] * (k' == p%16). No per-piece descriptors.
  * Per (tile c, group g): matmul psum[64,512] = Ag[:,g,c,:]^T @ BD.
    4 groups packed per [128,1024] psum tile; 2 psum tiles per tile c.
  * Evac psum -> bf16 SBUF (ACT + some DVE), one 512 KB out-DMA per tile
    (split across sync/scalar queues).
- Host: upcast bf16 -> f32, unshuffle token/emb order.
"""
import numpy as np
import ml_dtypes
from contextlib import ExitStack

import concourse.bass as bass
import concourse.bacc as bacc
import concourse.tile as tile
import concourse.mybir as mybir
from concourse import bass_utils

dt = mybir.dt
bf16 = ml_dtypes.bfloat16

R, M1, N1, M2, N2 = 8, 224, 64, 224, 32
VOCAB, EMB = M1 * M2, N1 * N2          # 50176, 2048
BATCH, SEQ = 4, 2048
NTOK = BATCH * SEQ                     # 8192
NCORES = 8
TPC = NTOK // NCORES                   # 1024 tokens per core
NTILES = TPC // 128                    # 8 tiles of 128 tokens
NG = 8                                 # groups per tile (token t = 8k+g)

_CACHE = {}


def _build():
    nc = bacc.Bacc("TRN2", num_devices=NCORES)
    A4 = nc.dram_tensor("A4", [M1, 512], dt.bfloat16, kind="ExternalInput")
    B4 = nc.dram_tensor("B4", [M2, 256], dt.bfloat16, kind="ExternalInput")
    idxA = nc.dram_tensor("idxA", [128, 64], dt.int16, kind="ExternalInput")
    idxB = nc.dram_tensor("idxB", [128, 64], dt.int16, kind="ExternalInput")
    maskT = nc.dram_tensor("maskT", [128, 512], dt.bfloat16, kind="ExternalInput")
    # DRAM scratch for the partition-shuffle round trip (host ignores);
    # separate tensors per half so Tile sees no false cross-half deps.
    Asc = [
        nc.dram_tensor(f"Asc{h}", [8, 16, 8, 4, 64], dt.bfloat16,
                       kind="ExternalOutput")
        for h in range(2)
    ]
    Bsc = [
        nc.dram_tensor(f"Bsc{h}", [8, 16, 8, 4, 32], dt.bfloat16,
                       kind="ExternalOutput")
        for h in range(2)
    ]
    out = nc.dram_tensor(
        "out", [NTILES, 128, 2048], dt.bfloat16, kind="ExternalOutput"
    )

    with tile.TileContext(nc) as tc, ExitStack() as ctx:
        cpool = ctx.enter_context(tc.tile_pool(name="const", bufs=1))
        ahp = ctx.enter_context(tc.tile_pool(name="ah", bufs=2))
        bhp = ctx.enter_context(tc.tile_pool(name="bh", bufs=2))
        agp = ctx.enter_context(tc.tile_pool(name="ag", bufs=2))
        bgp = ctx.enter_context(tc.tile_pool(name="bg", bufs=2))
        bdp = ctx.enter_context(tc.tile_pool(name="bd", bufs=4))
        psp = ctx.enter_context(tc.tile_pool(name="ps", bufs=4, space="PSUM"))
        evp = ctx.enter_context(tc.tile_pool(name="ev", bufs=3))

        idxA_sb = cpool.tile([128, 64], dt.int16, tag="idxA")
        idxB_sb = cpool.tile([128, 64], dt.int16, tag="idxB")
        mask_sb = cpool.tile([128, 512], dt.bfloat16, tag="mask")
        nc.sync.dma_start(idxA_sb[:], idxA[:])
        nc.sync.dma_start(idxB_sb[:], idxB[:])
        nc.sync.dma_start(mask_sb[:], maskT[:])

        # per half h (tiles 4h..4h+4, 512 tokens): gather token-major
        # (row idx[c*128+p] lands at out[p, c]), then round-trip shuffle.
        ag, bg = [], []
        for h in range(2):
            bh = bhp.tile([128, 4, 256], dt.bfloat16, tag="bh", name=f"bh{h}")
            nc.gpsimd.dma_gather(
                bh[:], B4[:], idxB_sb[:, 32 * h:32 * h + 32], 512, 512, 256
            )
            ah = ahp.tile([128, 4, 512], dt.bfloat16, tag="ah", name=f"ah{h}")
            nc.gpsimd.dma_gather(
                ah[:], A4[:], idxA_sb[:, 32 * h:32 * h + 32], 512, 512, 512
            )
            for s in range(8):
                nc.scalar.dma_start(
                    Bsc[h][s].rearrange("k g c a -> (k g) (c a)"),
                    bh[:, :, 32 * s:32 * s + 32],
                )
                nc.sync.dma_start(
                    Asc[h][s].rearrange("k g c a -> (k g) (c a)"),
                    ah[:, :, 64 * s:64 * s + 64],
                )
            # readbacks on sync (never behind the other half's writes,
            # which queue on scalar and become ready earlier)
            b = bgp.tile([128, NG, 4, 32], dt.bfloat16, tag="bg", name=f"bg{h}")
            nc.sync.dma_start(b[:], Bsc[h][:].rearrange("s k g c a -> (s k) g c a"))
            a = agp.tile([128, NG, 4, 64], dt.bfloat16, tag="ag", name=f"ag{h}")
            nc.sync.dma_start(a[:], Asc[h][:].rearrange("s k g c a -> (s k) g c a"))
            ag.append(a)
            bg.append(b)

        # per tile: 4 BD builds (DVE, 2 groups each) + 8 matmuls;
        # evac ACT + some DVE; out DMAs split sync/scalar.
        mask3 = mask_sb[:].rearrange("p (k b) -> p k b", k=16)
        for c in range(NTILES):
            h, cc = c // 4, c % 4
            ev = evp.tile([128, 2, 1024], dt.bfloat16, tag="ev")
            for half in range(2):
                ps = psp.tile([128, 1024], dt.float32, tag="ps")
                for qq in range(2):
                    g0 = 4 * half + 2 * qq
                    bdt = bdp.tile([128, 2, 16, 32], dt.bfloat16, tag="bd")
                    src = (
                        bg[h][:, g0:g0 + 2, cc, :]
                        .unsqueeze(2)
                        .broadcast_to([128, 2, 16, 32])
                    )
                    msk = mask3.unsqueeze(1).broadcast_to([128, 2, 16, 32])
                    nc.vector.tensor_mul(bdt[:], src, msk)
                    for j in range(2):
                        q = 2 * qq + j
                        nc.tensor.matmul(
                            ps[64 * (q % 2):64 * (q % 2) + 64,
                               512 * (q // 2):512 * (q // 2) + 512],
                            ag[h][:, g0 + j, cc, :],
                            bdt[:, j, :, :].rearrange("p k b -> p (k b)"),
                            start=True,
                            stop=True,
                        )
                if (2 * c + half) % 4 == 3:
                    nc.vector.tensor_copy(ev[:, half, :], ps[:])
                else:
                    nc.scalar.copy(ev[:, half, :], ps[:])
            oeng = nc.sync if c % 2 == 0 else nc.scalar
            oeng.dma_start(out[c], ev[:].rearrange("p h e -> p (h e)"))

    nc.compile()
    return nc


def _wrap_idxs(idx: np.ndarray) -> np.ndarray:
    """[n] -> SWDGE wrapped layout [128, n//16] int16; gather places
    row idx[c*128+p] at out[p, c]."""
    n = idx.shape[0]
    w = idx.reshape(n // 16, 16).T.astype(np.int16)
    return np.ascontiguousarray(np.tile(w, (8, 1)))


def _in_maps(A, B, x):
    A = np.asarray(A, dtype=np.float32)
    B = np.asarray(B, dtype=np.float32)
    xl = np.asarray(x).astype(np.int64).reshape(-1)           # [8192]

    A4 = np.ascontiguousarray(A.transpose(1, 0, 2).reshape(M1, 512)).astype(bf16)
    B4 = np.ascontiguousarray(B.transpose(1, 0, 2).reshape(M2, 256)).astype(bf16)

    i_all = (xl // M2).astype(np.int64)
    j_all = (xl % M2).astype(np.int64)

    # maskT[p, k'*32+b] = (k' == p % 16)
    mask = (np.arange(16)[None, :, None] == (np.arange(128) % 16)[:, None, None])
    maskT = np.ascontiguousarray(
        np.broadcast_to(mask, (128, 16, 32)).reshape(128, 512).astype(bf16)
    )

    in_maps = []
    for core in range(NCORES):
        sl = slice(core * TPC, (core + 1) * TPC)
        in_maps.append(
            dict(
                A4=A4,
                B4=B4,
                idxA=_wrap_idxs(i_all[sl]),
                idxB=_wrap_idxs(j_all[sl]),
                maskT=maskT,
            )
        )
    return in_maps


def _decode(res):
    outs = []
    for core in range(NCORES):
        o = np.asarray(res.results[core]["out"]).astype(np.float32)
        # out[c, p, inner]: p = 64*rowhalf + a (rowhalf = q%2);
        # inner = 1024*half + 512*colblk + 32*k + b  (g = 4*half+2*colblk+rowhalf)
        o = o.reshape(NTILES, 2, 64, 2, 2, 16, 32)  # c, rh, a, half, cb, k, b
        o = o.transpose(0, 5, 3, 4, 1, 2, 6)        # c, k, half, cb, rh, a, b
        outs.append(o.reshape(TPC, EMB))            # token = c*128 + 8k + g
    full = np.concatenate(outs, axis=0)             # [8192, 2048]
    return full.reshape(BATCH, SEQ, EMB).astype(np.float32)


def kernel(A: np.ndarray, B: np.ndarray, x: np.ndarray) -> np.ndarray:
    if "nc" not in _CACHE:
        _CACHE["nc"] = _build()
    nc = _CACHE["nc"]
    in_maps = _in_maps(A, B, x)
    res = bass_utils.run_bass_kernel_spmd(nc, in_maps, core_ids=list(range(NCORES)))
    return _decode(res)
